# revision 30
# baseline (speedup 1.0000x reference)
"""MHCN (multi-channel hypergraph GNN) Trainium2 kernel, 8-core SPMD.

Strategy: shard destination rows (users/items) across 8 cores. All SpMM
tables live in DRAM as fp16; each core gathers edge-source rows with
gpsimd.dma_gather, builds val-scaled one-hot matrices on DVE via fused
per-chunk tensor_scalar ((iota==rel)*val, 4x DVE mode), and segment-sums
via fp16 one-hot matmuls accumulated in fp32 PSUM. Gather calls span
multiple dest blocks (windows) to amortize SWDGE fixed cost. Updated
tables are AllGathered (fp16) between layers; the Tile scheduler
overlaps collectives and DMA with compute.
"""

import sys

sys.path.insert(0, "/opt/trn_rl_repo")

import numpy as np

import concourse.bacc as bacc
import concourse.bass as bass
import concourse.mybir as mybir
import concourse.tile as tile
from concourse import library_config
from concourse.bass_utils import run_bass_kernel_spmd

F32 = mybir.dt.float32
F16 = mybir.dt.float16
I16 = mybir.dt.int16

N_USERS, N_ITEMS, DIM = 50000, 25000, 128
NCORES = 8
U_PER, I_PER = N_USERS // NCORES, N_ITEMS // NCORES  # 6250, 3125
UBLK = (U_PER + 127) // 128  # 49 (last block 106 rows)
IBLK = (I_PER + 127) // 128  # 25 (last block 53 rows)
SPLIT = 32768  # int16 gather index limit
MAXCH = 64  # max chunks (x128 idxs) per dma_gather call
MAXGB = 7  # max dest blocks per gather window (psum residency bound)
BW = 5  # block batch width for stage/boundary/epilogue DMAs

# spmm job descriptors: (name, n_dest_blocks, split_src, has_val)
SPMMS = [
    ("h0", UBLK, True, True),
    ("h1", UBLK, True, True),
    ("h2", UBLK, True, True),
    ("ri", IBLK, True, False),  # R^T @ mixed -> items  (src = mixed, user-sized)
    ("ru", UBLK, False, False),  # R @ item_table -> users
]


def _prep_counts(rows, cols, base, ndest, split_src):
    m = (rows >= base) & (rows < base + ndest)
    r = rows[m] - base
    c = cols[m]
    blk = r >> 7
    half = (c >= SPLIT).astype(np.int64) if split_src else np.zeros_like(c, dtype=np.int64)
    nb = (ndest + 127) // 128
    cnt = np.bincount(blk * 2 + half, minlength=nb * 2).reshape(nb, 2)
    return cnt


def _slots(nb, gb, halves):
    """Stream order of (block, half) slots: window-major, half, block."""
    out = []
    for w0 in range(0, nb, gb):
        for h in range(halves):
            for b in range(w0, min(w0 + gb, nb)):
                out.append((b, h))
    return out


def _pick_gb(nch, halves):
    nb = nch.shape[0]
    for gb in range(MAXGB, 0, -1):
        ok = True
        for w0 in range(0, nb, gb):
            for h in range(halves):
                if int(nch[w0 : w0 + gb, h].sum()) > MAXCH:
                    ok = False
        if ok:
            return gb
    return 1


def _prep_fill(rows, cols, vals, base, ndest, split_src, nch, gb):
    """Padded chunk streams (idx16 [16, C*8], rel/val fp16) in window-major
    slot order, matching the common schedule nch."""
    m = (rows >= base) & (rows < base + ndest)
    r = rows[m] - base
    c = cols[m].astype(np.int64)
    v = vals[m].astype(np.float32)
    blk = r >> 7
    rel = (r & 127).astype(np.float32)
    half = (c >= SPLIT).astype(np.int64) if split_src else np.zeros_like(blk)
    nb = (ndest + 127) // 128
    halves = 2 if split_src else 1
    slots = _slots(nb, gb, halves)
    lookup = np.full(nb * 2, -1, np.int64)
    for i, (b, h) in enumerate(slots):
        lookup[b * 2 + h] = i
    sidx = lookup[blk * 2 + half]
    assert (sidx >= 0).all()
    order = np.argsort(sidx, kind="stable")
    s_sorted = sidx[order]
    cnt = np.bincount(s_sorted, minlength=len(slots))
    pad_cnt = np.array([int(nch[b, h]) for (b, h) in slots], np.int64) * 128
    assert (cnt <= pad_cnt).all()
    pad_start = np.concatenate([[0], np.cumsum(pad_cnt)[:-1]])
    grp_start = np.concatenate([[0], np.cumsum(cnt)[:-1]])
    within = np.arange(len(s_sorted)) - grp_start[s_sorted]
    pos = pad_start[s_sorted] + within
    L = int(pad_cnt.sum())
    idx = np.zeros(L, np.int64)
    rel_s = np.full(L, -1.0, np.float32)
    val_s = np.zeros(L, np.float32)
    idx[pos] = c[order] - half[order] * SPLIT
    rel_s[pos] = rel[order]
    val_s[pos] = v[order]
    C = L // 128
    idx16 = np.ascontiguousarray(idx.astype(np.int16).reshape(C * 8, 16).T)  # [16, C*8]
    relA = rel_s.reshape(C, 128).T.astype(np.float32)  # [128, C]
    valA = val_s.reshape(C, 128).T.astype(np.float32)
    return idx16, relA, valA


def _build_metadata(inp):
    edges = {
        "h0": (inp["Hs_row"], inp["Hs_col"], inp["Hs_val"], N_USERS, True),
        "h1": (inp["Hj_row"], inp["Hj_col"], inp["Hj_val"], N_USERS, True),
        "h2": (inp["Hp_row"], inp["Hp_col"], inp["Hp_val"], N_USERS, True),
        "ri": (inp["R_col"], inp["R_row"], inp["R_val"], N_ITEMS, True),
        "ru": (inp["R_row"], inp["R_col"], inp["R_val"], N_USERS, False),
    }
    sched = {}
    gbs = {}
    for s, (rows, cols, vals, ndest, split_src) in edges.items():
        per = ndest // NCORES
        cnts = [_prep_counts(rows, cols, cc * per, per, split_src) for cc in range(NCORES)]
        mx = np.maximum.reduce(cnts)
        nch = (mx + 127) // 128  # chunks per (block, half)
        assert nch.sum(axis=1).min() >= 1
        sched[s] = nch
        gbs[s] = _pick_gb(nch, 2 if split_src else 1)
    attv = (np.asarray(inp["attention_mat"], np.float32) @ np.asarray(inp["attention"], np.float32))
    attv_rep = np.tile(attv[None, :].astype(np.float16), (128, 1))
    gW = np.ascontiguousarray(np.asarray(inp["gating_W"], np.float32).astype(np.float16))
    gb_arr = np.asarray(inp["gating_b"], np.float32)
    has_bias = bool(np.any(gb_arr))
    gbias_rep = np.ascontiguousarray(
        np.tile(gb_arr[:, None, :], (1, 128, 1)).astype(np.float32)
    )
    u16 = np.asarray(inp["u_emb"], np.float32).astype(np.float16)
    i16 = np.asarray(inp["i_emb"], np.float32).astype(np.float16)
    in_maps = []
    for cc in range(NCORES):
        d = {
            "u16": np.ascontiguousarray(u16[cc * U_PER : (cc + 1) * U_PER]),
            "u16T": np.ascontiguousarray(u16[cc * U_PER : (cc + 1) * U_PER].T),
            "i16": np.ascontiguousarray(i16[cc * I_PER : (cc + 1) * I_PER]),
            "gW16": gW,
            "attv16": attv_rep,
        }
        if has_bias:
            d["gbias"] = gbias_rep
        for s, (rows, cols, vals, ndest, split_src) in edges.items():
            per = ndest // NCORES
            idx16, relA, valA = _prep_fill(
                rows, cols, vals, cc * per, per, split_src, sched[s], gbs[s]
            )
            d[s + "_idx"] = idx16
            if s in ("h0", "h1", "h2"):
                # interleave rel/val: [128, C, 2] -> [128, 2C]
                C = relA.shape[1]
                rv = np.empty((128, 2 * C), np.float32)
                rv[:, 0::2] = relA
                rv[:, 1::2] = valA
                d[s + "_rv"] = np.ascontiguousarray(rv)
            else:
                d[s + "_rel"] = np.ascontiguousarray(relA)
        in_maps.append(d)
    return in_maps, sched, gbs, has_bias


def _build_kernel(sched, gbs, has_bias):
    nc = bacc.Bacc("TRN2", target_bir_lowering=False, debug=False)
    AF = mybir.ActivationFunctionType
    ALU = mybir.AluOpType
    AX = mybir.AxisListType

    # ---- I/O ----
    P = {}
    P["u16"] = nc.declare_dram_parameter("u16", [U_PER, DIM], F16, isOutput=False)
    P["u16T"] = nc.declare_dram_parameter("u16T", [DIM, U_PER], F16, isOutput=False)
    P["i16"] = nc.declare_dram_parameter("i16", [I_PER, DIM], F16, isOutput=False)
    P["gW16"] = nc.declare_dram_parameter("gW16", [4, DIM, DIM], F16, isOutput=False)
    P["attv16"] = nc.declare_dram_parameter("attv16", [128, DIM], F16, isOutput=False)
    if has_bias:
        P["gbias"] = nc.declare_dram_parameter("gbias", [4, 128, DIM], F32, isOutput=False)
    CN = {}
    for s, nb, split_src, hasv in SPMMS:
        C = int(sched[s].sum())
        CN[s] = C
        P[s + "_idx"] = nc.declare_dram_parameter(s + "_idx", [16, C * 8], I16, isOutput=False)
        if hasv:
            P[s + "_rv"] = nc.declare_dram_parameter(s + "_rv", [128, 2 * C], F32, isOutput=False)
        else:
            P[s + "_rel"] = nc.declare_dram_parameter(s + "_rel", [128, C], F32, isOutput=False)
    out_u = nc.declare_dram_parameter("out_u", [U_PER, DIM], F32, isOutput=True)
    out_i = nc.declare_dram_parameter("out_i", [I_PER, DIM], F32, isOutput=True)

    # ---- internal DRAM ----
    rep_idx = {s: nc.dram_tensor(s + "_ridx", [128, CN[s] * 8], I16) for s, *_ in SPMMS}
    stage = {}
    for nm in ("cur0_l0", "cur1_l0", "cur2_l0", "mixed_l0",
               "cur0_l1", "cur1_l1", "cur2_l1", "mixed_l1", "cs_l1"):
        stage[nm] = nc.dram_tensor("stage_" + nm, [U_PER, DIM], F16)
    stage["item_l1"] = nc.dram_tensor("stage_item_l1", [I_PER, DIM], F16)
    stage["i16"] = nc.dram_tensor("stage_i16", [I_PER, DIM], F16)
    T = {}
    for nm in ("cur0_l0", "cur1_l0", "cur2_l0", "mixed_l0",
               "cur0_l1", "cur1_l1", "cur2_l1", "mixed_l1"):
        T[nm] = nc.dram_tensor("T_" + nm, [N_USERS, DIM], F16, addr_space="Shared")
    T["item_l1"] = nc.dram_tensor("T_item_l1", [N_ITEMS, DIM], F16, addr_space="Shared")
    T["i16"] = nc.dram_tensor("T_i16", [N_ITEMS, DIM], F16, addr_space="Shared")

    rg = [list(range(NCORES))]

    def dram_win(t, b0, nbw):
        """[128, nbw, 128] view of DRAM table rows [b0*128, (b0+nbw)*128)."""
        return t[b0 * 128 : (b0 + nbw) * 128].rearrange("(a p) m -> p a m", p=128)

    def sb_win(ap, m=128):
        return ap.rearrange("p (a m) -> p a m", m=m)

    with tile.TileContext(nc) as tc:
        with (
            tc.tile_pool(name="const", bufs=1) as cpool,
            tc.tile_pool(name="acc", bufs=1) as apool,
            tc.tile_pool(name="work", bufs=4) as wpool,
            tc.tile_pool(name="gat", bufs=3) as gpool,
            tc.tile_pool(name="oh", bufs=3) as opool,
            tc.tile_pool(name="idx", bufs=6) as ipool,
            tc.tile_pool(name="psum", bufs=8, space="PSUM") as ppool,
            tc.tile_pool(name="post", bufs=6) as spool,
        ):
            # constants
            gw_t = [cpool.tile([128, DIM], F16, tag=f"gw{c}", name=f"gw{c}") for c in range(4)]
            for c in range(4):
                nc.sync.dma_start(gw_t[c][:], P["gW16"][c])
            attv_t = cpool.tile([128, DIM], F16, tag="attv", name="attv")
            nc.sync.dma_start(attv_t[:], P["attv16"][:])
            bias_t = None
            if has_bias:
                bias_t = [cpool.tile([128, DIM], F32, tag=f"gb{c}", name=f"gb{c}") for c in range(4)]
                for c in range(4):
                    nc.sync.dma_start(bias_t[c][:], P["gbias"][c])
            iota_i = cpool.tile([128, 128], I16, tag="ioi", name="ioi")
            nc.gpsimd.iota(iota_i[:], pattern=[[1, 128]], base=0, channel_multiplier=0)
            iota_t = cpool.tile([128, 128], F16, tag="iof", name="iof")
            nc.vector.tensor_copy(out=iota_t[:], in_=iota_i[:])
            eps_t = cpool.tile([128, 1], F32, tag="eps", name="eps")
            nc.vector.memset(eps_t[:], 1e-12)

            # persistent accumulators (fp16, SBUF-resident)
            acc_c = [apool.tile([128, UBLK * 128], F16, tag=f"accc{k}", name=f"accc{k}") for k in range(3)]
            acc_s = apool.tile([128, UBLK * 128], F16, tag="accs", name="accs")
            acc_i = apool.tile([128, IBLK * 128], F16, tag="acci", name="acci")

            def ublk_rows(b):
                return min(128, U_PER - b * 128)

            def iblk_rows(b):
                return min(128, I_PER - b * 128)

            def replicate_idx(s, eng=None):
                eng = eng or nc.gpsimd
                srcf = P[s + "_idx"][:, :].rearrange("p w -> (p w)").unsqueeze(0)
                for k in range(8):
                    dstf = rep_idx[s][16 * k : 16 * (k + 1), :].rearrange("p w -> (p w)").unsqueeze(0)
                    eng.dma_start(dstf, srcf)

            for s in ("ru", "h0", "h1", "h2", "ri"):
                replicate_idx(s)

            # i_emb table allgather (earliest collective) + acc_i init
            nc.sync.dma_start(stage["i16"][:, :], P["i16"][:, :])

            import os

            def allgather(src, dst):
                if os.environ.get("KERNEL_NO_CC"):
                    nc.sync.dma_start(dst[: src.shape[0]], src[:])
                    return
                nc.gpsimd.collective_compute(
                    "AllGather",
                    ALU.bypass,
                    ins=[src[:]],
                    outs=[dst[:]],
                    replica_groups=rg,
                )

            allgather(stage["i16"], T["i16"])
            nc.sync.dma_start(
                sb_win(acc_i[:, : (IBLK - 1) * 128]), dram_win(P["i16"], 0, IBLK - 1)
            )
            lr = iblk_rows(IBLK - 1)
            nc.sync.dma_start(
                acc_i[:lr, (IBLK - 1) * 128 : (IBLK - 1) * 128 + DIM],
                P["i16"][(IBLK - 1) * 128 : (IBLK - 1) * 128 + lr],
            )

            def chan_att_mix(g, cs_tile, rows, mix_out):
                """mix_out[:rows] = sum_k softmax_k(w)*g[k] + cs_tile/2"""
                w = wpool.tile([128, 4], F32, tag="w", name="w")
                scr = wpool.tile([128, DIM], F16, tag="cascr", name="cascr")
                for k in range(3):
                    nc.vector.scalar_tensor_tensor(
                        out=scr[:rows], in0=g[k][:rows], scalar=1.0, in1=attv_t[:rows],
                        op0=ALU.mult, op1=ALU.mult, accum_out=w[:rows, k : k + 1],
                    )
                mx = wpool.tile([128, 1], F32, tag="mx", name="mx")
                nc.vector.tensor_reduce(out=mx[:rows], in_=w[:rows, :3], axis=AX.X, op=ALU.max)
                nc.vector.tensor_scalar(
                    out=w[:rows, :3], in0=w[:rows, :3], scalar1=mx[:rows], scalar2=None,
                    op0=ALU.subtract,
                )
                nc.scalar.activation(out=w[:rows, :3], in_=w[:rows, :3], func=AF.Exp)
                sm = wpool.tile([128, 1], F32, tag="sm", name="sm")
                nc.vector.tensor_reduce(out=sm[:rows], in_=w[:rows, :3], axis=AX.X, op=ALU.add)
                nc.vector.reciprocal(out=sm[:rows], in_=sm[:rows])
                nc.vector.tensor_scalar(
                    out=w[:rows, :3], in0=w[:rows, :3], scalar1=sm[:rows], scalar2=None,
                    op0=ALU.mult,
                )
                nc.vector.tensor_scalar(
                    out=mix_out[:rows], in0=cs_tile[:rows], scalar1=0.5, scalar2=None,
                    op0=ALU.mult,
                )
                for k in range(3):
                    nc.vector.scalar_tensor_tensor(
                        out=mix_out[:rows], in0=g[k][:rows], scalar=w[:rows, k : k + 1],
                        in1=mix_out[:rows], op0=ALU.mult, op1=ALU.add,
                    )

            def block_windows(nb):
                """(b0, nbw, full) windows: full-128-row blocks batched, the
                partial tail block alone."""
                full = nb - 1  # last block is partial for both UBLK and IBLK
                out = []
                b0 = 0
                while b0 < full:
                    out.append((b0, min(BW, full - b0), True))
                    b0 += BW
                out.append((full, 1, False))
                return out

            # ============ PROLOGUE pass A: gates into accumulators ============
            for b0, nbw, fullw in block_windows(UBLK):
                rowsw = 128 if fullw else ublk_rows(b0)
                ncols = (nbw - 1) * 128 + rowsw
                lhsT = wpool.tile([128, BW * 128], F16, tag="ulhsT", name="ulhsT")
                nc.sync.dma_start(
                    lhsT[:, :ncols], P["u16T"][:, b0 * 128 : b0 * 128 + ncols]
                )
                u_t = wpool.tile([128, BW * 128], F16, tag="urow", name="urow")
                if fullw:
                    nc.sync.dma_start(sb_win(u_t[:, : nbw * 128]), dram_win(P["u16"], b0, nbw))
                else:
                    nc.sync.dma_start(u_t[:rowsw, :128], P["u16"][b0 * 128 : b0 * 128 + rowsw])
                for i in range(nbw):
                    b = b0 + i
                    rows = ublk_rows(b)
                    for c in range(4):
                        ps = ppool.tile([128, DIM], F32, tag="psmm", name="psmm")
                        nc.tensor.matmul(
                            out=ps[:rows],
                            lhsT=lhsT[:, i * 128 : i * 128 + rows],
                            rhs=gw_t[c][:],
                            start=True, stop=True,
                        )
                        if has_bias:
                            nc.vector.tensor_tensor(
                                out=ps[:rows], in0=ps[:rows], in1=bias_t[c][:rows], op=ALU.add
                            )
                        sg = wpool.tile([128, DIM], F16, tag="sg", name="sg")
                        nc.scalar.activation(out=sg[:rows], in_=ps[:rows], func=AF.Sigmoid)
                        dst = acc_c[c] if c < 3 else acc_s
                        nc.vector.tensor_tensor(
                            out=dst[:rows, b * 128 : b * 128 + 128],
                            in0=u_t[:rows, i * 128 : (i + 1) * 128], in1=sg[:rows], op=ALU.mult,
                        )
            # stage gate tables for allgather (batched from accumulators)
            for b0, nbw, fullw in block_windows(UBLK):
                rows = 128 if fullw else ublk_rows(b0)
                for k in range(3):
                    if fullw:
                        nc.scalar.dma_start(
                            dram_win(stage[f"cur{k}_l0"], b0, nbw),
                            sb_win(acc_c[k][:, b0 * 128 : (b0 + nbw) * 128]),
                        )
                    else:
                        nc.scalar.dma_start(
                            stage[f"cur{k}_l0"][b0 * 128 : b0 * 128 + rows],
                            acc_c[k][:rows, b0 * 128 : b0 * 128 + 128],
                        )
            # ============ PROLOGUE pass B: mixed_l0 ============
            for b0, nbw, fullw in block_windows(UBLK):
                mixw = wpool.tile([128, BW * 128], F16, tag="mixw", name="mixw")
                for i in range(nbw):
                    b = b0 + i
                    rows = ublk_rows(b)
                    g = [acc_c[k][:, b * 128 : b * 128 + 128] for k in range(3)]
                    cs = acc_s[:, b * 128 : b * 128 + 128]
                    chan_att_mix(g, cs, rows, mixw[:, i * 128 : (i + 1) * 128])
                if fullw:
                    nc.scalar.dma_start(
                        dram_win(stage["mixed_l0"], b0, nbw), sb_win(mixw[:, : nbw * 128])
                    )
                else:
                    rows = ublk_rows(b0)
                    nc.scalar.dma_start(
                        stage["mixed_l0"][b0 * 128 : b0 * 128 + rows], mixw[:rows, :128]
                    )

            for k in range(3):
                allgather(stage[f"cur{k}_l0"], T[f"cur{k}_l0"])
            allgather(stage["mixed_l0"], T["mixed_l0"])

            # ================= SPMM =================
            def spmm(s, nb, split_src, hasv, src_tbl, rowfn, stage_to, acc_to):
                nch = sched[s]
                gb = gbs[s]
                halves = 2 if split_src else 1
                ch_cursor = 0
                for w0 in range(0, nb, gb):
                    wblocks = list(range(w0, min(w0 + gb, nb)))
                    half_data = []  # (h, n, oh_tile, per-block cnts)
                    for h in range(halves):
                        cnts = [int(nch[b, h]) for b in wblocks]
                        n = sum(cnts)
                        ch0 = ch_cursor
                        ch_cursor += n
                        if n == 0:
                            half_data.append((h, 0, None, cnts))
                            continue
                        assert n <= MAXCH
                        idx_t = ipool.tile([128, MAXCH * 8], I16, tag="idx", name="idx")
                        nc.sync.dma_start(
                            idx_t[:, : n * 8], rep_idx[s][:, ch0 * 8 : (ch0 + n) * 8]
                        )
                        if hasv:
                            rv_t = ipool.tile([128, MAXCH * 2], F32, tag="rv", name="rv")
                            nc.sync.dma_start(
                                rv_t[:, : n * 2], P[s + "_rv"][:, ch0 * 2 : (ch0 + n) * 2]
                            )
                        else:
                            rv_t = ipool.tile([128, MAXCH], F32, tag="rel", name="rel")
                            nc.sync.dma_start(rv_t[:, :n], P[s + "_rel"][:, ch0 : ch0 + n])
                        G = gpool.tile([128, MAXCH * 128], F16, tag="G", name="G")
                        src = src_tbl[SPLIT:, :] if h == 1 else src_tbl[:, :]
                        if os.environ.get("KERNEL_NO_GATHER"):
                            nc.vector.memset(G[:, : n * 128], 0.0)
                        else:
                            nc.gpsimd.dma_gather(
                                G[:, : n * 128].rearrange("p (n m) -> p n m", m=128),
                                src,
                                idx_t[:, : n * 8],
                                n * 128,
                                n * 128,
                                DIM,
                                single_packet=False,
                            )
                        oh = opool.tile([128, MAXCH * 128], F16, tag="oh", name="oh")
                        for c in range(n):
                            if hasv:
                                nc.vector.tensor_scalar(
                                    out=oh[:, c * 128 : (c + 1) * 128],
                                    in0=iota_t[:],
                                    scalar1=rv_t[:, 2 * c : 2 * c + 1],
                                    scalar2=rv_t[:, 2 * c + 1 : 2 * c + 2],
                                    op0=ALU.is_equal, op1=ALU.mult,
                                )
                            else:
                                nc.vector.tensor_scalar(
                                    out=oh[:, c * 128 : (c + 1) * 128],
                                    in0=iota_t[:],
                                    scalar1=rv_t[:, c : c + 1],
                                    scalar2=None,
                                    op0=ALU.is_equal,
                                )
                        half_data.append((h, n, (oh, G), cnts))
                    # matmuls: half-major, block within half; psum per block
                    ps_of = {}
                    lasth = {}
                    for b in wblocks:
                        hs = [h for h in range(halves) if int(nch[b, h]) > 0]
                        assert hs, f"block {b} of {s} has no chunks"
                        lasth[b] = hs[-1]
                    for h, n, tiles, cnts in half_data:
                        if n == 0:
                            continue
                        oh, G = tiles
                        off = 0
                        for b_i, b in enumerate(wblocks):
                            cnt = cnts[b_i]
                            for j in range(cnt):
                                if b not in ps_of:
                                    ps_of[b] = ppool.tile([128, DIM], F32, tag="psmm", name="psmm")
                                    first = True
                                else:
                                    first = False
                                nc.tensor.matmul(
                                    out=ps_of[b][:],
                                    lhsT=oh[:, (off + j) * 128 : (off + j + 1) * 128],
                                    rhs=G[:, (off + j) * 128 : (off + j + 1) * 128],
                                    start=first,
                                    stop=(h == lasth[b] and j == cnt - 1),
                                )
                            off += cnt
                    # finalize blocks in window
                    for b in wblocks:
                        rows = rowfn(b)
                        ps = ps_of[b]
                        sq = spool.tile([128, DIM], F16, tag="sq", name="sq")
                        ss = spool.tile([128, 1], F32, tag="ss", name="ss")
                        nc.scalar.activation(
                            out=sq[:], in_=ps[:], func=AF.Square, accum_out=ss[:]
                        )
                        # sqrt(ss + eps) ~= sqrt(max(ss, eps)) for ss >= 0
                        nc.scalar.activation(out=ss[:], in_=ss[:], func=AF.Sqrt, bias=eps_t[:])
                        nc.vector.reciprocal(out=ss[:], in_=ss[:])
                        nt = spool.tile([128, DIM], F16, tag="nt16", name="nt16")
                        nc.scalar.activation(out=nt[:], in_=ps[:], func=AF.Copy, scale=ss[:])
                        nc.vector.tensor_tensor(
                            out=acc_to[:, b * 128 : b * 128 + 128],
                            in0=acc_to[:, b * 128 : b * 128 + 128],
                            in1=nt[:], op=ALU.add,
                        )
                        if stage_to is not None:
                            st = spool.tile([128, DIM], F16, tag="st16", name="st16")
                            nc.scalar.activation(out=st[:rows], in_=ps[:rows], func=AF.Copy)
                            nc.sync.dma_start(stage_to[b * 128 : b * 128 + rows], st[:rows])

            # ================= LAYER 0 =================
            spmm("ru", UBLK, False, False, T["i16"], ublk_rows, stage["cs_l1"], acc_s)
            for k in range(3):
                spmm(f"h{k}", UBLK, True, True, T[f"cur{k}_l0"], ublk_rows,
                     stage[f"cur{k}_l1"], acc_c[k])
                allgather(stage[f"cur{k}_l1"], T[f"cur{k}_l1"])
            spmm("ri", IBLK, True, False, T["mixed_l0"], iblk_rows, stage["item_l1"], acc_i)
            allgather(stage["item_l1"], T["item_l1"])

            # boundary: mixed_l1 from staged layer-1 outputs (batched loads)
            for b0, nbw, fullw in block_windows(UBLK):
                gws = []
                for k in range(3):
                    gt = wpool.tile([128, BW * 128], F16, tag=f"bg{k}", name=f"bg{k}")
                    if fullw:
                        nc.scalar.dma_start(sb_win(gt[:, : nbw * 128]), dram_win(stage[f"cur{k}_l1"], b0, nbw))
                    else:
                        rows = ublk_rows(b0)
                        nc.scalar.dma_start(gt[:rows, :128], stage[f"cur{k}_l1"][b0 * 128 : b0 * 128 + rows])
                    gws.append(gt)
                csw = wpool.tile([128, BW * 128], F16, tag="bcs", name="bcs")
                if fullw:
                    nc.scalar.dma_start(sb_win(csw[:, : nbw * 128]), dram_win(stage["cs_l1"], b0, nbw))
                else:
                    rows = ublk_rows(b0)
                    nc.scalar.dma_start(csw[:rows, :128], stage["cs_l1"][b0 * 128 : b0 * 128 + rows])
                mixw = wpool.tile([128, BW * 128], F16, tag="bmixw", name="bmixw")
                for i in range(nbw):
                    b = b0 + i
                    rows = ublk_rows(b)
                    g = [gws[k][:, i * 128 : (i + 1) * 128] for k in range(3)]
                    chan_att_mix(g, csw[:, i * 128 : (i + 1) * 128], rows,
                                 mixw[:, i * 128 : (i + 1) * 128])
                if fullw:
                    nc.scalar.dma_start(dram_win(stage["mixed_l1"], b0, nbw), sb_win(mixw[:, : nbw * 128]))
                else:
                    rows = ublk_rows(b0)
                    nc.scalar.dma_start(stage["mixed_l1"][b0 * 128 : b0 * 128 + rows], mixw[:rows, :128])
            allgather(stage["mixed_l1"], T["mixed_l1"])

            # ================= LAYER 1 =================
            for k in range(3):
                spmm(f"h{k}", UBLK, True, True, T[f"cur{k}_l1"], ublk_rows, None, acc_c[k])
            spmm("ru", UBLK, False, False, T["item_l1"], ublk_rows, None, acc_s)
            spmm("ri", IBLK, True, False, T["mixed_l1"], iblk_rows, None, acc_i)

            # ================= EPILOGUE =================
            for b0, nbw, fullw in block_windows(UBLK):
                mixw = wpool.tile([128, BW * 128], F32, tag="emixw", name="emixw")
                for i in range(nbw):
                    b = b0 + i
                    rows = ublk_rows(b)
                    g = [acc_c[k][:, b * 128 : b * 128 + 128] for k in range(3)]
                    cs = acc_s[:, b * 128 : b * 128 + 128]
                    chan_att_mix(g, cs, rows, mixw[:, i * 128 : (i + 1) * 128])
                if fullw:
                    nc.scalar.dma_start(dram_win(out_u, b0, nbw), sb_win(mixw[:, : nbw * 128]))
                else:
                    rows = ublk_rows(b0)
                    nc.scalar.dma_start(out_u[b0 * 128 : b0 * 128 + rows], mixw[:rows, :128])
            for b0, nbw, fullw in block_windows(IBLK):
                t32 = wpool.tile([128, BW * 128], F32, tag="eit", name="eit")
                if fullw:
                    nc.vector.tensor_copy(
                        out=t32[:, : nbw * 128], in_=acc_i[:, b0 * 128 : (b0 + nbw) * 128]
                    )
                    nc.scalar.dma_start(dram_win(out_i, b0, nbw), sb_win(t32[:, : nbw * 128]))
                else:
                    rows = iblk_rows(b0)
                    nc.vector.tensor_copy(
                        out=t32[:rows, :128], in_=acc_i[:rows, b0 * 128 : b0 * 128 + 128]
                    )
                    nc.scalar.dma_start(out_i[b0 * 128 : b0 * 128 + rows], t32[:rows, :128])

    nc.compile()
    return nc


def kernel(**inputs):
    inputs = {k: np.asarray(v) for k, v in inputs.items()}
    in_maps, sched, gbs, has_bias = _build_metadata(inputs)
    nc = _build_kernel(sched, gbs, has_bias)
    import os, time as _t
    res = run_bass_kernel_spmd(nc, in_maps, list(range(NCORES)))
    if os.environ.get("KERNEL_TRACE"):
        # no NTFF hook in this container: report wall time of a second,
        # already-compiled execution as an upper bound on HW exec time
        t0 = _t.time()
        res = run_bass_kernel_spmd(nc, in_maps, list(range(NCORES)))
        kernel.last_exec_time_ns = int((_t.time() - t0) * 1e9)
    out = np.zeros((N_USERS + N_ITEMS, DIM), np.float32)
    for cc in range(NCORES):
        out[cc * U_PER : (cc + 1) * U_PER] = res.results[cc]["out_u"]
        out[N_USERS + cc * I_PER : N_USERS + (cc + 1) * I_PER] = res.results[cc]["out_i"]
    return out


if __name__ == "__main__":
    pass
